# revision 1
# baseline (speedup 1.0000x reference)
"""Trainium2 Bass kernel for nn_AttentionHeads (PaiNN-style GNN edge attention).

Computes, per edge e with endpoints (i, j) = nbrs[e]:
    q = W_q @ x_i[i]; k = W_k @ x_i[j]           (per-head linears)
    dk = silu(W_dk @ feats(dist[e]))              (RBF * cosine envelope)
    weights[e, h] = silu(sum_f q*k*dk)

Strategy (8 NeuronCores, data-parallel over edges):
  - x table stored in SBUF as fp16 rows [x | x] (256B); xi / xj fetched with
    SBUF-source dma_gather(transpose=True), landing directly in the
    [feature_partition, edge_free] layout the TensorEngine needs.
  - RBF features come from a host-precomputed table over 16384 distance bins
    (pure function of the bin grid), also gathered transposed; the 20 feature
    rows are replicated at partition offsets 0/32/64/96 so the four W_dk
    channel-chunk matmuls can row-tile.
  - Per 512-edge group: Q/K matmuls row-tiled (rows 0-63 / 64-127), dkpre
    matmuls at 32-row strips, z=q*k (DVE), dk=silu(dkpre) (ACT, bias folds
    b_dk), p=z*dk (DVE fp16 2x), head-reduction via mask matmuls accumulated
    in PSUM, final silu (ACT) into an fp16 [8, E] output buffer.
"""

import numpy as np

N_NODES = 20000
N_EDGES = 150000
FEAT = 64
HEADS = 8
N_RBF = 20
CUTOFF = 5.0

N_CORES = 8
GROUP = 512                    # edges per compute group
NGROUP = 37                    # groups per core
EC = GROUP * NGROUP            # padded edges per core = 18944
E_BASE = N_EDGES // N_CORES    # real edges per core = 18750
NBINS = 16384                  # distance bins for the feats table
NODE_PAD = 20096               # nodes padded to multiple of 128 (157 ranks)
CH = 4                         # channel chunks of 128 (= 2 heads each)
QUARTER_GROUPS = (10, 9, 9, 9)  # gather pipelining split
ACT_FN = "Silu"  # overridden to Sigmoid by the CoreSim test (sim lacks Silu)
COPY_PATTERN = "mix"  # qk PSUM->SBUF drain engine: "mix" | "act" | "dve"


def _silu(v):
    return v / (1.0 + np.exp(-v))


def _feats_of(d):
    # [len(d), N_RBF] float64: sin(n*pi*d/cutoff)/d * cosine envelope
    n = np.arange(1, N_RBF + 1, dtype=np.float64)
    s = np.sin(n * np.pi * d[:, None] / CUTOFF) / d[:, None]
    env = np.where(d < CUTOFF, 0.5 * (np.cos(np.pi * d / CUTOFF) + 1.0), 0.0)
    return s * env[:, None]


def _wrap_idx(idx):
    # dma_gather index layout: position i -> partition i%16, free i//16,
    # replicated across the 8 Q7 core groups (128 partitions total).
    n = idx.shape[0]
    w = idx.reshape(n // 16, 16).T.astype(np.int16)  # [16, n//16]
    return np.ascontiguousarray(np.tile(w, (8, 1)))  # [128, n//16]


def _table_sbuf_layout(rows):
    # rows: [n_tokens, 128] (n_tokens multiple of 128). SBUF-source gather
    # reads token t of rank r from partition t, free bytes [r*256, r*256+256).
    n = rows.shape[0]
    assert n % 128 == 0 and rows.shape[1] == 128
    return np.ascontiguousarray(
        rows.reshape(n // 128, 128, 128).transpose(1, 0, 2).reshape(128, n)
    )


_PROGRAM_CACHE = {}


def _build_program(with_qk_bias):
    import concourse.tile as tile
    from concourse import bacc, mybir

    key = (bool(with_qk_bias), ACT_FN, COPY_PATTERN, EC)
    if key in _PROGRAM_CACHE:
        return _PROGRAM_CACHE[key]

    f16 = mybir.dt.float16
    f32 = mybir.dt.float32
    i16 = mybir.dt.int16
    AF = mybir.ActivationFunctionType
    AF_FN = getattr(AF, ACT_FN)

    nc = bacc.Bacc("TRN2", target_bir_lowering=False, debug=False)

    xtab_d = nc.dram_tensor("xtab", [128, NODE_PAD], f16, kind="ExternalInput")
    ftab_d = nc.dram_tensor("ftab", [128, NBINS], f16, kind="ExternalInput")
    wqk_d = nc.dram_tensor("wqk", [128, 512], f16, kind="ExternalInput")
    wdk_d = nc.dram_tensor("wdk", [128, 512], f16, kind="ExternalInput")
    mask_d = nc.dram_tensor("mask4", [128, 32], f16, kind="ExternalInput")
    bdk_d = nc.dram_tensor("bdk", [128, 4], f32, kind="ExternalInput")
    idxi_d = nc.dram_tensor("idxi", [128, EC // 16], i16, kind="ExternalInput")
    idxj_d = nc.dram_tensor("idxj", [128, EC // 16], i16, kind="ExternalInput")
    idxb_d = nc.dram_tensor("idxb", [128, EC // 16], i16, kind="ExternalInput")
    if with_qk_bias:
        bqk_d = nc.dram_tensor("bqk", [128, 8], f32, kind="ExternalInput")
    wout_d = nc.dram_tensor("wout", [8, EC], f16, kind="ExternalOutput")

    with tile.TileContext(nc) as tc:
        with (
            tc.tile_pool(name="tabs", bufs=1) as tabs,
            tc.tile_pool(name="gath", bufs=2) as gath,
            tc.tile_pool(name="work", bufs=4) as work,
            tc.tile_pool(name="outp", bufs=1) as outp,
            tc.tile_pool(name="psum", bufs=2, space="PSUM") as psum,
        ):
            # ---- resident tables / constants ----
            xtab = tabs.tile([128, NODE_PAD], f16)
            ftab = tabs.tile([128, NBINS], f16)
            wqk = tabs.tile([128, 512], f16)
            wdk = tabs.tile([128, 512], f16)
            mask4 = tabs.tile([128, 32], f16)
            bdk = tabs.tile([128, 4], f32)
            idxi = tabs.tile([128, EC // 16], i16)
            idxj = tabs.tile([128, EC // 16], i16)
            idxb = tabs.tile([128, EC // 16], i16)
            w_all = outp.tile([8, EC], f16)

            # x-table + its idx lists first: the xi/xj gathers depend only
            # on these, so they can start while the feats table still loads
            nc.sync.dma_start(xtab[:], xtab_d[:])
            nc.sync.dma_start(idxi[:], idxi_d[:])
            nc.sync.dma_start(idxj[:], idxj_d[:])
            nc.sync.dma_start(wqk[:], wqk_d[:])
            nc.sync.dma_start(wdk[:], wdk_d[:])
            nc.sync.dma_start(mask4[:], mask_d[:])
            nc.sync.dma_start(bdk[:], bdk_d[:])
            nc.sync.dma_start(ftab[:], ftab_d[:])
            nc.sync.dma_start(idxb[:], idxb_d[:])
            if with_qk_bias:
                bqk = tabs.tile([128, 8], f32)
                nc.sync.dma_start(bqk[:], bqk_d[:])

            GCHUNK = 2048  # idxs per dma_gather call (ring capacity limit)

            def one_gather(dst, table, idx, idx_col0, o, m):
                nc.gpsimd.dma_gather(
                    dst[:, :, o : o + m],
                    table[:],
                    idx[:, idx_col0 + o // 16 : idx_col0 + (o + m) // 16],
                    num_idxs=m,
                    num_idxs_reg=m,
                    elem_size=128,
                    transpose=True,
                    sbuf_tokens_per_rank=128,
                    sbuf_free_dim_per_rank=256,
                    sbuf_free_dim_pad_per_rank=0,
                    sbuf_byte_offset=0,
                    single_packet=False,
                )

            g0 = 0
            for qi, qgroups in enumerate(QUARTER_GROUPS):
                qe = qgroups * GROUP          # edges this quarter
                e0 = g0 * GROUP               # first edge of quarter
                c0, c1 = e0 // 16, (e0 + qe) // 16
                xiT = gath.tile([128, 1, QUARTER_GROUPS[0] * GROUP], f16, tag="xiT")
                xjT = gath.tile([128, 1, QUARTER_GROUPS[0] * GROUP], f16, tag="xjT")
                ftT = gath.tile([128, 1, QUARTER_GROUPS[0] * GROUP], f16, tag="ftT")
                for tdst, ttab, tidx in (
                    (xiT, xtab, idxi), (xjT, xtab, idxj), (ftT, ftab, idxb)
                ):
                    for o in range(0, qe, GCHUNK):
                        m = min(GCHUNK, qe - o)
                        one_gather(tdst, ttab, tidx, c0, o, m)

                for g in range(qgroups):
                    gg = g0 + g               # global group index
                    s = g * GROUP             # edge offset within quarter
                    w_ps = psum.tile([8, GROUP], f32, tag="w")
                    for c in range(CH):
                        qk_ps = psum.tile([128, 2 * GROUP], f32, tag="qk")
                        d_ps = psum.tile([128, GROUP], f32, tag="d")
                        cs = slice(c * 128, (c + 1) * 128)
                        nc.tensor.matmul(
                            qk_ps[:, 0:GROUP],
                            wqk[0:64, cs],
                            xiT[0:64, 0, s : s + GROUP],
                        )
                        nc.tensor.matmul(
                            qk_ps[:, GROUP : 2 * GROUP],
                            wqk[64:128, cs],
                            xjT[64:128, 0, s : s + GROUP],
                        )
                        nc.tensor.matmul(
                            d_ps[:],
                            wdk[32 * c : 32 * c + N_RBF, cs],
                            ftT[32 * c : 32 * c + N_RBF, 0, s : s + GROUP],
                            tile_position=(32 * c, 0),
                        )
                        if with_qk_bias:
                            nc.vector.tensor_scalar_add(
                                qk_ps[:, 0:GROUP], qk_ps[:, 0:GROUP],
                                bqk[:, c : c + 1],
                            )
                            nc.vector.tensor_scalar_add(
                                qk_ps[:, GROUP : 2 * GROUP],
                                qk_ps[:, GROUP : 2 * GROUP],
                                bqk[:, 4 + c : 5 + c],
                            )
                        kc_sb = work.tile([128, GROUP], f32, tag="kc")
                        z_sb = work.tile([128, GROUP], f16, tag="z")
                        dk_sb = work.tile([128, GROUP], f16, tag="dk")
                        p_sb = work.tile([128, GROUP], f16, tag="p")
                        # drain k PSUM->SBUF; mostly ACT (DVE is the busier
                        # engine: z at PSUM-1x + p)
                        use_act = (gg * CH + c) % 3 != 2 if COPY_PATTERN == "mix" \
                            else COPY_PATTERN == "act"
                        if use_act:
                            nc.scalar.copy(kc_sb[:], qk_ps[:, GROUP : 2 * GROUP])
                        else:
                            nc.vector.tensor_copy(
                                kc_sb[:], qk_ps[:, GROUP : 2 * GROUP]
                            )
                        nc.vector.tensor_mul(
                            z_sb[:], qk_ps[:, 0:GROUP], kc_sb[:]
                        )
                        nc.scalar.activation(
                            dk_sb[:], d_ps[:], AF_FN, bias=bdk[:, c : c + 1]
                        )
                        nc.vector.tensor_mul(p_sb[:], z_sb[:], dk_sb[:])
                        nc.tensor.matmul(
                            w_ps[:],
                            mask4[:, 8 * c : 8 * c + 8],
                            p_sb[:],
                            start=(c == 0),
                            stop=(c == CH - 1),
                            skip_group_check=True,
                        )
                    nc.scalar.activation(
                        w_all[:, gg * GROUP : (gg + 1) * GROUP], w_ps[:], AF_FN
                    )
                g0 += qgroups

            nc.sync.dma_start(wout_d[:], w_all[:])

    nc.compile()
    _PROGRAM_CACHE[key] = nc
    return nc


def _prep_inputs(dist, nbrs, x_i, W_q, b_q, W_k, b_k, W_dk, b_dk):
    f16 = np.float16
    # x table: fp16 rows [x | x], padded to NODE_PAD tokens
    xd = np.zeros((NODE_PAD, 128), f16)
    xh = x_i.astype(f16)
    xd[:N_NODES, :64] = xh
    xd[:N_NODES, 64:] = xh
    xtab = _table_sbuf_layout(xd)

    # feats table over NBINS distance bins, 20 cols replicated at 0/32/64/96
    hbin = (CUTOFF - 0.5) / (NBINS - 1)
    dgrid = 0.5 + hbin * np.arange(NBINS)
    fg = _feats_of(dgrid).astype(f16)  # [NBINS, 20]
    fd = np.zeros((NBINS, 128), f16)
    for s in range(4):
        fd[:, 32 * s : 32 * s + N_RBF] = fg
    ftab = _table_sbuf_layout(fd)

    # weights in lhsT layout [f_in, h*64+g]
    wqk = np.zeros((128, 512), f16)
    wqk[:64] = W_q.transpose(2, 0, 1).reshape(64, 512).astype(f16)
    wqk[64:] = W_k.transpose(2, 0, 1).reshape(64, 512).astype(f16)
    wdk_full = W_dk.transpose(2, 0, 1).reshape(N_RBF, 512).astype(f16)
    wdk = np.zeros((128, 512), f16)
    for c in range(CH):
        wdk[32 * c : 32 * c + N_RBF, 128 * c : 128 * (c + 1)] = wdk_full[
            :, 128 * c : 128 * (c + 1)
        ]

    # head-reduction masks: chunk c covers heads 2c (rows 0-63), 2c+1 (64-127)
    mask4 = np.zeros((128, 32), f16)
    for c in range(CH):
        mask4[0:64, 8 * c + 2 * c] = 1.0
        mask4[64:128, 8 * c + 2 * c + 1] = 1.0

    bdk = np.zeros((128, 4), np.float32)
    for c in range(CH):
        bdk[0:64, c] = b_dk[2 * c]
        bdk[64:128, c] = b_dk[2 * c + 1]

    with_qk_bias = bool(np.any(b_q) or np.any(b_k))
    bqk = None
    if with_qk_bias:
        bqk = np.zeros((128, 8), np.float32)
        for c in range(CH):
            bqk[0:64, c] = b_q[2 * c]
            bqk[64:128, c] = b_q[2 * c + 1]
            bqk[0:64, 4 + c] = b_k[2 * c]
            bqk[64:128, 4 + c] = b_k[2 * c + 1]

    hb = (CUTOFF - 0.5) / (NBINS - 1)
    bins_all = np.clip(np.round((dist - 0.5) / hb), 0, NBINS - 1).astype(np.int64)

    in_maps = []
    for c in range(N_CORES):
        lo = c * E_BASE
        ii = np.zeros(EC, np.int64)
        jj = np.zeros(EC, np.int64)
        bb = np.zeros(EC, np.int64)
        ii[:E_BASE] = nbrs[lo : lo + E_BASE, 0]
        jj[:E_BASE] = nbrs[lo : lo + E_BASE, 1]
        bb[:E_BASE] = bins_all[lo : lo + E_BASE]
        m = {
            "xtab": xtab,
            "ftab": ftab,
            "wqk": wqk,
            "wdk": wdk,
            "mask4": mask4,
            "bdk": bdk,
            "idxi": _wrap_idx(ii),
            "idxj": _wrap_idx(jj),
            "idxb": _wrap_idx(bb),
        }
        if with_qk_bias:
            m["bqk"] = bqk
        in_maps.append(m)
    return in_maps, with_qk_bias


def kernel(dist, nbrs, x_i, W_q, b_q, W_k, b_k, W_dk, b_dk):
    from concourse.bass_utils import run_bass_kernel_spmd

    in_maps, with_qk_bias = _prep_inputs(
        np.asarray(dist), np.asarray(nbrs), np.asarray(x_i),
        np.asarray(W_q), np.asarray(b_q), np.asarray(W_k), np.asarray(b_k),
        np.asarray(W_dk), np.asarray(b_dk),
    )
    nc = _build_program(with_qk_bias)
    res = run_bass_kernel_spmd(nc, in_maps, list(range(N_CORES))).results

    out = np.empty((N_EDGES, HEADS), np.float32)
    for c in range(N_CORES):
        w = res[c]["wout"]  # [8, EC] fp16
        out[c * E_BASE : (c + 1) * E_BASE] = w[:, :E_BASE].T.astype(np.float32)
    return out



# revision 3
# speedup vs baseline: 1.5302x; 1.5302x over previous
"""Trainium2 Bass kernel for nn_AttentionHeads (PaiNN-style GNN edge attention).

Computes, per edge e with endpoints (i, j) = nbrs[e]:
    q = W_q @ x_i[i]; k = W_k @ x_i[j]            (per-head linears)
    dk = silu(W_dk @ feats(dist[e]) + b_dk)       (RBF * cosine envelope)
    weights[e, h] = silu(sum_f q*k*dk)

Strategy (8 NeuronCores, data-parallel over edges):
  - dk is a pure function of dist, so the whole silu(W_dk @ feats + b) vector
    (512 values) is tabulated on the host over 16384 distance bins and fetched
    per edge with one DRAM-source dma_gather (elem_size=512 -> 1KB tokens, no
    small-transfer penalty).  This removes the dk matmuls and dk silu entirely.
  - xi / xj fetched with DRAM-source dma_gather(transpose=True) from a
    [N_NODES, 128] fp16 table of [x | x] rows, landing in the
    [feature_partition, edge_free] layout the TensorEngine needs.
  - Per 512-edge group, per 128-channel chunk: Q/K matmuls into one PSUM tile
    [q | k]; the PSUM drain + q*k*dk product is routed per-chunk over three
    patterns to balance DVE vs ACT (PSUM can only be read by one operand of a
    tensor_tensor):
      R1: kc = ACT copy(k); z = DVE mul(q_ps, kc); p = DVE mul(z, dk)
      R2: qkd = ACT copy(q|k); z = DVE mul(qkd_q, qkd_k); p = DVE mul(z, dk)
      R3: qd = DVE mul(q_ps, dk); p = DVE mul(k_ps, qd)
  - Head-reduction via mask matmuls accumulated in PSUM, final silu (ACT)
    into fp16 [8, wave] output tiles, written back per 2048-edge wave.
"""

import numpy as np

N_NODES = 20000
N_EDGES = 150000
FEAT = 64
HEADS = 8
N_RBF = 20
CUTOFF = 5.0

N_CORES = 8
GROUP = 512                    # edges per compute group
NGROUP = 37                    # groups per core
EC = GROUP * NGROUP            # padded edges per core = 18944
E_BASE = N_EDGES // N_CORES    # real edges per core = 18750
NBINS = 16384                  # distance bins for the dk table
WAVE_GROUPS = 4                # groups per gather/compute wave
ACT_FN = "Silu"

# Per-chunk drain route, indexed by [group % len][chunk]. 1/2/3 = R1/R2/R3.
# Mix tuned so DVE ~= ACT busy (R1: ACT 570 + DVE 851; R2: ACT 996 + DVE 386;
# R3: DVE 1316).
ROUTE_PATTERN = [
    (3, 1, 1, 2),
    (3, 1, 2, 2),
    (3, 1, 2, 2),
    (3, 2, 2, 1),
    (3, 1, 2, 2),
]


def _silu(v):
    return v / (1.0 + np.exp(-v))


def _feats_of(d):
    # [len(d), N_RBF] float64: sin(n*pi*d/cutoff)/d * cosine envelope
    n = np.arange(1, N_RBF + 1, dtype=np.float64)
    s = np.sin(n * np.pi * d[:, None] / CUTOFF) / d[:, None]
    env = np.where(d < CUTOFF, 0.5 * (np.cos(np.pi * d / CUTOFF) + 1.0), 0.0)
    return s * env[:, None]


def _wrap_idx(idx):
    # dma_gather index layout: position i -> partition i%16, free i//16,
    # replicated across the 8 Q7 core groups (128 partitions total).
    n = idx.shape[0]
    w = idx.reshape(n // 16, 16).T.astype(np.int16)  # [16, n//16]
    return np.ascontiguousarray(np.tile(w, (8, 1)))  # [128, n//16]


_PROGRAM_CACHE = {}


def _waves():
    out = []
    e0 = 0
    while e0 < EC:
        ne = min(WAVE_GROUPS * GROUP, EC - e0)
        out.append((e0, ne))
        e0 += ne
    return out


def _build_program(with_qk_bias):
    import concourse.tile as tile
    from concourse import bacc, mybir

    key = (bool(with_qk_bias), ACT_FN, tuple(map(tuple, ROUTE_PATTERN)), EC)
    if key in _PROGRAM_CACHE:
        return _PROGRAM_CACHE[key]

    f16 = mybir.dt.float16
    f32 = mybir.dt.float32
    i16 = mybir.dt.int16
    AF = mybir.ActivationFunctionType
    AF_FN = getattr(AF, ACT_FN)

    nc = bacc.Bacc("TRN2", target_bir_lowering=False, debug=False)

    xtab_d = nc.dram_tensor("xtab", [N_NODES, 128], f16, kind="ExternalInput")
    dktab_d = nc.dram_tensor("dktab", [NBINS, 512], f16, kind="ExternalInput")
    wqk_d = nc.dram_tensor("wqk", [128, 512], f16, kind="ExternalInput")
    mask_d = nc.dram_tensor("mask4", [128, 32], f16, kind="ExternalInput")
    idxi_d = nc.dram_tensor("idxi", [128, EC // 16], i16, kind="ExternalInput")
    idxj_d = nc.dram_tensor("idxj", [128, EC // 16], i16, kind="ExternalInput")
    idxd_d = nc.dram_tensor("idxd", [128, EC // 16], i16, kind="ExternalInput")
    if with_qk_bias:
        bqk_d = nc.dram_tensor("bqk", [128, 8], f32, kind="ExternalInput")
    wout_d = nc.dram_tensor("wout", [8, EC], f16, kind="ExternalOutput")

    WV = WAVE_GROUPS * GROUP

    with tile.TileContext(nc) as tc:
        with (
            tc.tile_pool(name="tabs", bufs=1) as tabs,
            tc.tile_pool(name="gath", bufs=2) as gath,
            tc.tile_pool(name="work", bufs=4) as work,
            tc.tile_pool(name="outp", bufs=2) as outp,
            tc.tile_pool(name="psum", bufs=3, space="PSUM") as psum,
            tc.tile_pool(name="psumw", bufs=2, space="PSUM") as psumw,
        ):
            idxi = tabs.tile([128, EC // 16], i16)
            idxj = tabs.tile([128, EC // 16], i16)
            idxd = tabs.tile([128, EC // 16], i16)
            wqk = tabs.tile([128, 512], f16)
            mask4 = tabs.tile([128, 32], f16)

            nc.sync.dma_start(idxi[:], idxi_d[:])
            nc.sync.dma_start(idxj[:], idxj_d[:])
            nc.sync.dma_start(idxd[:], idxd_d[:])
            nc.sync.dma_start(wqk[:], wqk_d[:])
            nc.sync.dma_start(mask4[:], mask_d[:])
            if with_qk_bias:
                bqk = tabs.tile([128, 8], f32)
                nc.sync.dma_start(bqk[:], bqk_d[:])

            gg = 0  # global group index
            for e0, ne in _waves():
                c0 = e0 // 16
                xiT = gath.tile([128, 1, ne], f16, tag=f"xiT{ne}")
                xjT = gath.tile([128, 1, ne], f16, tag=f"xjT{ne}")
                dkT = gath.tile([128, 4, ne], f16, tag=f"dkT{ne}")
                nc.gpsimd.dma_gather(
                    xiT[:, :, :], xtab_d[:], idxi[:, c0 : c0 + ne // 16],
                    num_idxs=ne, num_idxs_reg=ne, elem_size=128,
                    transpose=True, single_packet=False,
                )
                nc.gpsimd.dma_gather(
                    xjT[:, :, :], xtab_d[:], idxj[:, c0 : c0 + ne // 16],
                    num_idxs=ne, num_idxs_reg=ne, elem_size=128,
                    transpose=True, single_packet=False,
                )
                nc.gpsimd.dma_gather(
                    dkT[:, :, :], dktab_d[:], idxd[:, c0 : c0 + ne // 16],
                    num_idxs=ne, num_idxs_reg=ne, elem_size=512,
                    transpose=True, single_packet=False,
                )

                w_wave = outp.tile([8, WV], f16, tag="w_wave")
                for g in range(ne // GROUP):
                    s = g * GROUP
                    routes = ROUTE_PATTERN[gg % len(ROUTE_PATTERN)]
                    w_ps = psumw.tile([8, GROUP], f32, tag="w")
                    for c in range(4):
                        cs = slice(c * 128, (c + 1) * 128)
                        qk_ps = psum.tile([128, 2 * GROUP], f32, tag="qk")
                        q_ap = qk_ps[:, 0:GROUP]
                        k_ap = qk_ps[:, GROUP : 2 * GROUP]
                        nc.tensor.matmul(q_ap, wqk[0:64, cs],
                                         xiT[0:64, 0, s : s + GROUP])
                        nc.tensor.matmul(k_ap, wqk[64:128, cs],
                                         xjT[64:128, 0, s : s + GROUP])
                        if with_qk_bias:
                            nc.vector.tensor_scalar_add(q_ap, q_ap,
                                                        bqk[:, c : c + 1])
                            nc.vector.tensor_scalar_add(k_ap, k_ap,
                                                        bqk[:, 4 + c : 5 + c])
                        dk_ap = dkT[:, c, s : s + GROUP]
                        p_sb = work.tile([128, GROUP], f16, tag="p")
                        r = routes[c]
                        if r == 1:
                            kc = work.tile([128, GROUP], f16, tag="kc")
                            z = work.tile([128, GROUP], f16, tag="z")
                            nc.scalar.copy(kc[:], k_ap)
                            nc.vector.tensor_mul(z[:], q_ap, kc[:])
                            nc.vector.tensor_mul(p_sb[:], z[:], dk_ap)
                        elif r == 2:
                            qkd = work.tile([128, 2 * GROUP], f16, tag="qkd")
                            z = work.tile([128, GROUP], f16, tag="z")
                            nc.scalar.copy(qkd[:], qk_ps[:])
                            nc.vector.tensor_mul(z[:], qkd[:, 0:GROUP],
                                                 qkd[:, GROUP : 2 * GROUP])
                            nc.vector.tensor_mul(p_sb[:], z[:], dk_ap)
                        else:
                            qd = work.tile([128, GROUP], f16, tag="qd")
                            nc.vector.tensor_mul(qd[:], q_ap, dk_ap)
                            nc.vector.tensor_mul(p_sb[:], k_ap, qd[:])
                        nc.tensor.matmul(
                            w_ps[:], mask4[:, 8 * c : 8 * c + 8], p_sb[:],
                            start=(c == 0), stop=(c == 3),
                            skip_group_check=True,
                        )
                    nc.scalar.activation(
                        w_wave[:, s : s + GROUP], w_ps[:], AF_FN
                    )
                    gg += 1
                nc.sync.dma_start(wout_d[:, e0 : e0 + ne], w_wave[:, 0:ne])

    nc.compile()
    _PROGRAM_CACHE[key] = nc
    return nc


def _prep_inputs(dist, nbrs, x_i, W_q, b_q, W_k, b_k, W_dk, b_dk):
    f16 = np.float16
    # x table: fp16 rows [x | x]
    xtab = np.empty((N_NODES, 128), f16)
    xh = x_i.astype(f16)
    xtab[:, :64] = xh
    xtab[:, 64:] = xh

    # dk table over NBINS distance bins: row layout chunk-major, within a
    # chunk heads (2c, 2c+1) each spanning 64 features.
    hbin = (CUTOFF - 0.5) / (NBINS - 1)
    dgrid = 0.5 + hbin * np.arange(NBINS)
    fg = _feats_of(dgrid)  # [NBINS, 20] float64
    pre = fg @ W_dk.reshape(HEADS * FEAT, N_RBF).astype(np.float64).T
    pre += b_dk.reshape(-1).astype(np.float64)
    dktab = _silu(pre).astype(f16)  # [NBINS, 512] in (h, f) order
    # reorder columns from (h*64+f) to chunk-major (c*128 + (h%2)*64 + f)
    order = np.empty(512, np.int64)
    for c in range(4):
        order[c * 128 : c * 128 + 64] = (2 * c) * 64 + np.arange(64)
        order[c * 128 + 64 : c * 128 + 128] = (2 * c + 1) * 64 + np.arange(64)
    dktab = np.ascontiguousarray(dktab[:, order])

    # weights in lhsT layout [f_in, h*64+g]
    wqk = np.zeros((128, 512), f16)
    wqk[:64] = W_q.transpose(2, 0, 1).reshape(64, 512).astype(f16)
    wqk[64:] = W_k.transpose(2, 0, 1).reshape(64, 512).astype(f16)

    # head-reduction masks: chunk c covers heads 2c (rows 0-63), 2c+1 (64-127)
    mask4 = np.zeros((128, 32), f16)
    for c in range(4):
        mask4[0:64, 8 * c + 2 * c] = 1.0
        mask4[64:128, 8 * c + 2 * c + 1] = 1.0

    with_qk_bias = bool(np.any(b_q) or np.any(b_k))
    bqk = None
    if with_qk_bias:
        bqk = np.zeros((128, 8), np.float32)
        for c in range(4):
            bqk[0:64, c] = b_q[2 * c]
            bqk[64:128, c] = b_q[2 * c + 1]
            bqk[0:64, 4 + c] = b_k[2 * c]
            bqk[64:128, 4 + c] = b_k[2 * c + 1]

    bins_all = np.clip(np.round((dist - 0.5) / hbin), 0, NBINS - 1).astype(np.int64)

    in_maps = []
    for c in range(N_CORES):
        lo = c * E_BASE
        ii = np.zeros(EC, np.int64)
        jj = np.zeros(EC, np.int64)
        bb = np.zeros(EC, np.int64)
        ii[:E_BASE] = nbrs[lo : lo + E_BASE, 0]
        jj[:E_BASE] = nbrs[lo : lo + E_BASE, 1]
        bb[:E_BASE] = bins_all[lo : lo + E_BASE]
        m = {
            "xtab": xtab,
            "dktab": dktab,
            "wqk": wqk,
            "mask4": mask4,
            "idxi": _wrap_idx(ii),
            "idxj": _wrap_idx(jj),
            "idxd": _wrap_idx(bb),
        }
        if with_qk_bias:
            m["bqk"] = bqk
        in_maps.append(m)
    return in_maps, with_qk_bias


def kernel(dist, nbrs, x_i, W_q, b_q, W_k, b_k, W_dk, b_dk):
    from concourse.bass_utils import run_bass_kernel_spmd

    in_maps, with_qk_bias = _prep_inputs(
        np.asarray(dist), np.asarray(nbrs), np.asarray(x_i),
        np.asarray(W_q), np.asarray(b_q), np.asarray(W_k), np.asarray(b_k),
        np.asarray(W_dk), np.asarray(b_dk),
    )
    nc = _build_program(with_qk_bias)
    res = run_bass_kernel_spmd(nc, in_maps, list(range(N_CORES))).results

    out = np.empty((N_EDGES, HEADS), np.float32)
    for c in range(N_CORES):
        w = res[c]["wout"]  # [8, EC] fp16
        out[c * E_BASE : (c + 1) * E_BASE] = w[:, :E_BASE].T.astype(np.float32)
    return out
